# revision 10
# baseline (speedup 1.0000x reference)
"""Trainium2 Bass kernel for AttentionLayer (B=4, S=1024, D=1024, H=16).

Sharding: pure data-parallel over (batch, query-half) = 8 shards, one per
NeuronCore. No collectives. Each core computes, for its batch b and query
block Q (512 queries), the full 16-head attention + output projection.

Device-side layout is "transposed activation" space: the host pre-transposes
q/k/v/weights/mask so every tensor arrives with the contraction dimension on
SBUF partitions and the device never transposes anything:

  qhT = (Wq @ qT) / 32        [d_out, Sq]    (lhsT = WqT, rhs = qT)
  khT = Wk @ kT               [d_out, Sk]
  vh  = v @ WvT               [Sk, d_out]    (lhsT = vT,  rhs = WvT)
  sT_h = khT_h^T @ qhT_h      [Sk, Sq]       per head, K = 64
  u_h  = sT_h - 1e9 * maskT   (DVE, reads PSUM)
  p_h  = exp(u_h)             (ACT)
  ctxT_h / sums = vh_aug^T @ p_h   [65, Sq]  (ones column gives softmax sums)
  attnT_h = p_h * recip(sums) (recip broadcast across partitions via K=1 matmul)
  outT = WdT^T @ ctxT + bd    [d_out, Sq]

Outputs (attnT per head [Sk, Sq], outT [d_out, Sq]) are un-transposed on the
host during the gather step.
"""

import os

os.environ.setdefault("MYCRO_LOCAL_CACHE", "1")
import sys

if "/opt/trn_rl_repo" not in sys.path:
    sys.path.insert(0, "/opt/trn_rl_repo")

from contextlib import ExitStack

import numpy as np

import concourse.bass as bass
import concourse.mybir as mybir
import concourse.tile as tile
from concourse import bacc
from concourse.bass_utils import run_bass_kernel_spmd

B, S, D = 4, 1024, 1024
H, DH = 16, 64
SQ = 512  # queries per core (half a batch)
NCORES = 8
KT = 8  # 1024 / 128 k-tiles (also d tiles)
F32 = mybir.dt.float32
BF16 = mybir.dt.bfloat16
I32 = mybir.dt.int32

LAST_RESULT = None  # BassKernelResults from the most recent run (for test.py)
TRACE_TMPDIR = None  # set by test.py to persist NEFF/ntff/perfetto artifacts


def _build():
    nc = bacc.Bacc("TRN2", target_bir_lowering=False, debug=False, num_devices=NCORES)

    qT_d = nc.dram_tensor("qT", [D, SQ], F32, kind="ExternalInput")
    kT_d = nc.dram_tensor("kT", [D, S], F32, kind="ExternalInput")
    vT_d = nc.dram_tensor("vT", [D, S], F32, kind="ExternalInput")
    maskT_d = nc.dram_tensor("maskT", [S, SQ], I32, kind="ExternalInput")
    wqT_d = nc.dram_tensor("wqT", [D, D], F32, kind="ExternalInput")
    wkT_d = nc.dram_tensor("wkT", [D, D], F32, kind="ExternalInput")
    wvT_d = nc.dram_tensor("wvT", [D, D], F32, kind="ExternalInput")
    wdT_d = nc.dram_tensor("wdT", [D, D], F32, kind="ExternalInput")
    bd_d = nc.dram_tensor("bd2", [128, 8], F32, kind="ExternalInput")

    attnT_d = nc.dram_tensor("attnT", [H, S, SQ], F32, kind="ExternalOutput")
    outT_d = nc.dram_tensor("outT", [D, SQ], F32, kind="ExternalOutput")

    # DRAM views with the 1024-long partition axis split into 8 x 128
    qT_v = qT_d.ap().rearrange("(a p) n -> p a n", p=128)
    kT_v = kT_d.ap().rearrange("(a p) n -> p a n", p=128)
    vT_v = vT_d.ap().rearrange("(a p) n -> p a n", p=128)
    maskT_v = maskT_d.ap().rearrange("(a p) n -> p a n", p=128)
    wqT_v = wqT_d.ap().rearrange("(a p) n -> p a n", p=128)
    wkT_v = wkT_d.ap().rearrange("(a p) n -> p a n", p=128)
    wvT_v = wvT_d.ap().rearrange("(a p) n -> p a n", p=128)
    wdT_v = wdT_d.ap().rearrange("(a p) n -> p a n", p=128)

    with tile.TileContext(nc) as tc, ExitStack() as top:
        # ---- persistent tiles ----
        persist = top.enter_context(tc.tile_pool(name="persist", bufs=1))
        qhT_sb = persist.tile([128, KT, SQ], BF16)   # [d_out, Sq] all heads
        khT_sb = persist.tile([128, KT, S], BF16)    # [d_out, Sk]
        vh_sb = persist.tile([128, KT, H * (DH + 1)], BF16)  # [Sk, 16*(64+1)] aug
        maskf_sb = persist.tile([128, KT, SQ], F32)  # [Sk, Sq] float mask
        ctxT_sb = persist.tile([128, KT, SQ], BF16)  # [d_in, Sq]
        bd_sb = persist.tile([128, 8], F32)
        ones_sb = persist.tile([128, 128], F32)

        nc.gpsimd.memset(ones_sb[:], 1.0)
        nc.sync.dma_start(bd_sb[:], bd_d[:])
        # mask: int32 -> f32 cast during DMA (SWDGE)
        nc.gpsimd.dma_start(maskf_sb[:], maskT_v[:])
        # ones column LAST (c=64): softmax sums land on PSUM partition 64,
        # ctx rows on partitions 0..63 (engine APs need 32-aligned start)
        vh_aug4 = vh_sb.rearrange("p a (h c) -> p a h c", c=DH + 1)
        nc.gpsimd.memset(vh_aug4[:, :, :, DH], 1.0)

        # ---- phase A: Q projection ----
        with tc.tile_pool(name="projq", bufs=1) as pool, tc.tile_pool(
            name="psA", bufs=4, space=bass.MemorySpace.PSUM
        ) as psA:
            wq_sb = pool.tile([128, KT, D], BF16)
            qT_sb = pool.tile([128, KT, SQ], BF16)
            nc.gpsimd.dma_start(wq_sb[:], wqT_v[:])  # f32 -> bf16 cast
            nc.gpsimd.dma_start(qT_sb[:], qT_v[:])
            for m in range(KT):
                ps = psA.tile([128, SQ], F32, tag="ps")
                for kk in range(KT):
                    nc.tensor.matmul(
                        ps[:],
                        wq_sb[:, kk, m * 128 : (m + 1) * 128],
                        qT_sb[:, kk, :],
                        start=(kk == 0),
                        stop=(kk == KT - 1),
                    )
                # copy + scale by 1/sqrt(D); cast to bf16
                nc.scalar.activation(
                    qhT_sb[:, m, :],
                    ps[:],
                    mybir.ActivationFunctionType.Identity,
                    scale=float(1.0 / np.sqrt(D)),
                )

        # ---- phase B: K projection ----
        with tc.tile_pool(name="projk", bufs=1) as pool, tc.tile_pool(
            name="psB", bufs=4, space=bass.MemorySpace.PSUM
        ) as psB:
            wk_sb = pool.tile([128, KT, D], BF16)
            kT_sb = pool.tile([128, KT, S], BF16)
            nc.gpsimd.dma_start(wk_sb[:], wkT_v[:])
            nc.gpsimd.dma_start(kT_sb[:], kT_v[:])
            for m in range(KT):
                for n in range(2):
                    ps = psB.tile([128, 512], F32, tag="ps")
                    for kk in range(KT):
                        nc.tensor.matmul(
                            ps[:],
                            wk_sb[:, kk, m * 128 : (m + 1) * 128],
                            kT_sb[:, kk, n * 512 : (n + 1) * 512],
                            start=(kk == 0),
                            stop=(kk == KT - 1),
                        )
                    nc.scalar.activation(
                        khT_sb[:, m, n * 512 : (n + 1) * 512],
                        ps[:],
                        mybir.ActivationFunctionType.Identity,
                    )

        # ---- phase C: V projection (normal orientation: vh[s, d]) ----
        with tc.tile_pool(name="projv", bufs=1) as pool, tc.tile_pool(
            name="psC", bufs=4, space=bass.MemorySpace.PSUM
        ) as psC:
            wv_sb = pool.tile([128, KT, D], BF16)
            vT_sb = pool.tile([128, KT, S], BF16)
            nc.gpsimd.dma_start(wv_sb[:], wvT_v[:])
            nc.gpsimd.dma_start(vT_sb[:], vT_v[:])
            for m in range(KT):  # s tile
                for n in range(2):  # d_out 512-chunk = 8 heads
                    ps = psC.tile([128, 512], F32, tag="ps")
                    for kk in range(KT):
                        nc.tensor.matmul(
                            ps[:],
                            vT_sb[:, kk, m * 128 : (m + 1) * 128],
                            wv_sb[:, kk, n * 512 : (n + 1) * 512],
                            start=(kk == 0),
                            stop=(kk == KT - 1),
                        )
                    # write into the strided 64-col blocks of vh_aug
                    ps4 = ps.rearrange("p (h c) -> p h c", c=DH)
                    nc.scalar.activation(
                        vh_aug4[:, m, n * 8 : (n + 1) * 8, 0:DH],
                        ps4[:],
                        mybir.ActivationFunctionType.Identity,
                    )

        # ---- phase D: attention per head ----
        with tc.tile_pool(name="attnw", bufs=2) as work, tc.tile_pool(
            name="attno", bufs=4
        ) as outp, tc.tile_pool(
            name="psS", bufs=4, space=bass.MemorySpace.PSUM
        ) as psS, tc.tile_pool(
            name="psX", bufs=2, space=bass.MemorySpace.PSUM
        ) as psX, tc.tile_pool(
            name="psR", bufs=2, space=bass.MemorySpace.PSUM
        ) as psR:
            for h in range(H):
                hp, ho = h // 2, (h % 2) * 64  # d_out tile / partition offset
                p_h = work.tile([128, KT, SQ], BF16, tag="p")
                for j in range(KT):
                    sps = psS.tile([128, SQ], F32, tag="sps")
                    nc.tensor.matmul(
                        sps[:],
                        khT_sb[ho : ho + 64, hp, j * 128 : (j + 1) * 128],
                        qhT_sb[ho : ho + 64, hp, :],
                        start=True,
                        stop=True,
                    )
                    # u = s + (-1e9)*mask  (DVE, PSUM + SBUF -> SBUF bf16)
                    nc.vector.scalar_tensor_tensor(
                        p_h[:, j, :],
                        maskf_sb[:, j, :],
                        -1.0e9,
                        sps[:],
                        op0=mybir.AluOpType.mult,
                        op1=mybir.AluOpType.add,
                    )
                # p = exp(u) in place, one big ACT op
                nc.scalar.activation(
                    p_h[:, :, :], p_h[:, :, :], mybir.ActivationFunctionType.Exp
                )
                # ctxT_aug[65, SQ]: rows 0..63 = ctx, row 64 = softmax sums
                xps = psX.tile([DH + 1, SQ], F32, tag="xps")
                for j in range(KT):
                    nc.tensor.matmul(
                        xps[:],
                        vh_sb[:, j, h * (DH + 1) : (h + 1) * (DH + 1)],
                        p_h[:, j, :],
                        start=(j == 0),
                        stop=(j == KT - 1),
                    )
                # reciprocal of sums (partition 64), broadcast to all 128
                # partitions via a K=1 outer-product matmul (row 64 of ones)
                recip_s = outp.tile([DH + 1, SQ], F32, tag="recip")
                nc.vector.reciprocal(recip_s[DH : DH + 1, :], xps[DH : DH + 1, :])
                rps = psR.tile([128, SQ], F32, tag="rps")
                nc.tensor.matmul(
                    rps[:],
                    ones_sb[DH : DH + 1, :],
                    recip_s[DH : DH + 1, :],
                    start=True,
                    stop=True,
                )
                # stage broadcast reciprocal in SBUF (tensor_tensor may read
                # at most one PSUM operand)
                rsb = outp.tile([128, SQ], F32, tag="rsb")
                nc.scalar.activation(
                    rsb[:], rps[:], mybir.ActivationFunctionType.Identity
                )
                # normalized ctx rows into ctxT (partition shift via DMA)
                ctxn = outp.tile([DH, SQ], BF16, tag="ctxn")
                nc.vector.tensor_tensor(
                    ctxn[:], xps[0:DH, :], rsb[0:DH, :], mybir.AluOpType.mult
                )
                nc.sync.dma_start(ctxT_sb[ho : ho + DH, hp, :], ctxn[:])
                # normalized attention probabilities -> HBM
                for j in range(KT):
                    attn_sb = outp.tile([128, SQ], F32, tag="attn")
                    nc.vector.tensor_tensor(
                        attn_sb[:], p_h[:, j, :], rsb[:], mybir.AluOpType.mult
                    )
                    nc.sync.dma_start(
                        attnT_d[h, j * 128 : (j + 1) * 128, :], attn_sb[:]
                    )

        # ---- phase E: output projection ----
        with tc.tile_pool(name="projo", bufs=1) as pool, tc.tile_pool(
            name="outw", bufs=4
        ) as outw, tc.tile_pool(
            name="psE", bufs=4, space=bass.MemorySpace.PSUM
        ) as psE:
            wd_sb = pool.tile([128, KT, D], BF16)
            nc.gpsimd.dma_start(wd_sb[:], wdT_v[:])
            for m in range(KT):
                ps = psE.tile([128, SQ], F32, tag="ps")
                for kk in range(KT):
                    nc.tensor.matmul(
                        ps[:],
                        wd_sb[:, kk, m * 128 : (m + 1) * 128],
                        ctxT_sb[:, kk, :],
                        start=(kk == 0),
                        stop=(kk == KT - 1),
                    )
                o_sb = outw.tile([128, SQ], F32, tag="o")
                nc.scalar.activation(
                    o_sb[:],
                    ps[:],
                    mybir.ActivationFunctionType.Identity,
                    bias=bd_sb[:, m : m + 1],
                )
                nc.sync.dma_start(outT_d[m * 128 : (m + 1) * 128, :], o_sb[:])

    nc.compile()
    return nc


_NC_CACHE = None


def _get_nc():
    global _NC_CACHE
    if _NC_CACHE is None:
        _NC_CACHE = _build()
    return _NC_CACHE


def kernel(q, k, v, mask, Wq, Wk, Wv, Wd, bd):
    q = np.asarray(q, dtype=np.float32)
    k = np.asarray(k, dtype=np.float32)
    v = np.asarray(v, dtype=np.float32)
    mask = np.asarray(mask)
    Wq = np.asarray(Wq, dtype=np.float32)
    Wk = np.asarray(Wk, dtype=np.float32)
    Wv = np.asarray(Wv, dtype=np.float32)
    Wd = np.asarray(Wd, dtype=np.float32)
    bd = np.asarray(bd, dtype=np.float32)

    wqT = np.ascontiguousarray(Wq.T)
    wkT = np.ascontiguousarray(Wk.T)
    wvT = np.ascontiguousarray(Wv.T)
    wdT = np.ascontiguousarray(Wd.T)
    bd2 = np.ascontiguousarray(bd.reshape(8, 128).T)

    in_maps = []
    for core in range(NCORES):
        b, half = core // 2, core % 2
        qs = slice(half * SQ, (half + 1) * SQ)
        in_maps.append(
            {
                "qT": np.ascontiguousarray(q[b].T[:, qs]),
                "kT": np.ascontiguousarray(k[b].T),
                "vT": np.ascontiguousarray(v[b].T),
                "maskT": np.ascontiguousarray(mask[b, 0].T[:, qs]).astype(np.int32),
                "wqT": wqT,
                "wkT": wkT,
                "wvT": wvT,
                "wdT": wdT,
                "bd2": bd2,
            }
        )

    nc = _get_nc()
    res = run_bass_kernel_spmd(
        nc, in_maps, core_ids=list(range(NCORES)), tmpdir=TRACE_TMPDIR
    )
    global LAST_RESULT
    LAST_RESULT = res

    out = np.empty((B, S, D), dtype=np.float32)
    attn = np.empty((B, H, S, S), dtype=np.float32)
    for core in range(NCORES):
        b, half = core // 2, core % 2
        qs = slice(half * SQ, (half + 1) * SQ)
        out[b, qs, :] = res.results[core]["outT"].T
        attn[b, :, qs, :] = res.results[core]["attnT"].transpose(0, 2, 1)
    return out, attn
